# revision 1
# baseline (speedup 1.0000x reference)
"""HSIC loss kernel for Trainium2, 8 NeuronCores.

Math: for each feature column c of X [2048, 16], K_c = rbf kernel matrix
(zero diag). Output = sum over feature pairs a<b of squared unbiased-HSIC
combination of T[a,b]=sum(K_a*K_b), rowsums A, total sums S.

Device strategy (SPMD over 8 cores, sharded over sample rows i):
  core r owns i in [r*256, (r+1)*256). Tiles are [128 j-partitions, 256 i-free].
  exponent(-gamma_c*(x_i-x_j)^2) built as:
     VE scalar_tensor_tensor: E = (xi - 2*xj)*xi = xi^2 - 2*xi*xj   (per-partition scalar 2*xj)
     ACT activation:          K = Exp(E*(-g_c) + (-g_c*xj^2)), accum_out -> per-j partial colsum
  K written bf16; TensorE accumulates the 16x16 Gram (trace) matrix over all
  (i,j) positions via [128, 16feat*8i] self-matmuls into one PSUM tile.
  Host combines per-core partials in float64.
"""

import sys
import numpy as np

if "/opt/trn_rl_repo" not in sys.path:
    sys.path.insert(0, "/opt/trn_rl_repo")

N = 2048
D = 16
P = 128
NCORES = 8
NI = N // NCORES          # 256 rows of i per core
NJB = N // P              # 16 j-blocks of 128 partitions
GJ = 8                    # i's packed per gram matmul -> D*GJ = 128 cols

_NC_CACHE = {}


def _patch_tile_drain():
    """Walrus in this container accepts only 1 sync-wait per instruction.
    Tile routinely attaches several. Hoist extra waits onto single-wait NoOp
    carriers emitted just before the instruction on the same engine, and
    split the tail drain's per-engine waits the same way."""
    import concourse.mybir as mybir
    import concourse.tile as tile_mod
    from concourse.vector_clock import ScopedClock, VectorClock

    if getattr(tile_mod.TileContext, "_drain_patched", False):
        return

    orig_add = tile_mod.TileContext._add_instruction
    counter = [0]

    def _add_instruction(self, inst):
        si = inst.sync_info
        if si is not None and si.on_wait is not None and len(si.on_wait) > 1:
            waits = list(si.on_wait)
            for w in waits[:-1]:
                counter[0] += 1
                carrier = mybir.InstNoOp(name=f"waitc-{counter[0]}")
                carrier.engine = inst.engine
                carrier.sync_info = mybir.SyncInfo(on_wait=[w], on_update=[])
                orig_add(self, carrier)
            inst.sync_info = mybir.SyncInfo(
                on_wait=[waits[-1]], on_update=list(si.on_update or [])
            )
        orig_add(self, inst)

    def _drain_and_barrier(self, tick_clock, wait_clock):
        vec = list(tick_clock.global_clock)
        for i, v in enumerate(vec):
            if v <= 0:
                continue
            sub = [v if j == i else 0 for j in range(len(vec))]
            carrier = self.nc.sync.nop(nofuse=True)
            wait_clock.add_sem_waits(
                carrier.ins, ScopedClock({None: VectorClock(sub)})
            )
        self.nc.sync.drain()
        self.nc.all_engine_barrier()
        popped = self.nc._tile_sem_poison_stack.pop()
        assert popped is self._sem_poison
        self.nc.clear_and_free_semaphores(list(self.sems.allocated().values()))
        self.nc.all_engine_barrier()

    tile_mod.TileContext._add_instruction = _add_instruction
    tile_mod.TileContext._drain_and_barrier = _drain_and_barrier
    tile_mod.TileContext._drain_patched = True


def _build_nc():
    import concourse.bass as bass
    import concourse.mybir as mybir
    from concourse.tile import TileContext

    _patch_tile_drain()

    f32 = mybir.dt.float32
    bf16 = mybir.dt.bfloat16

    nc = bass.Bass("TRN2")
    xi_rep_d = nc.dram_tensor("xi_rep", [P, D * NI], f32, kind="ExternalInput")
    sc2xj_d = nc.dram_tensor("sc2xj", [P, NJB * D], f32, kind="ExternalInput")
    bias_d = nc.dram_tensor("bias_t", [P, NJB * D], f32, kind="ExternalInput")
    gam_d = nc.dram_tensor("gam_t", [P, D], f32, kind="ExternalInput")
    apart_d = nc.dram_tensor("apart", [P, NJB * D], f32, kind="ExternalOutput")
    tpart_d = nc.dram_tensor("tpart", [P, P], f32, kind="ExternalOutput")

    n_mm = NJB * (NI // GJ)

    with TileContext(nc) as tc:
        with (
            tc.tile_pool(name="const", bufs=1) as cpool,
            tc.tile_pool(name="e", bufs=4) as epool,
            tc.tile_pool(name="k", bufs=2) as kpool,
            tc.tile_pool(name="ps", bufs=1, space="PSUM") as pspool,
        ):
            xi_rep = cpool.tile([P, D * NI], f32)
            sc2xj = cpool.tile([P, NJB * D], f32)
            bias_sb = cpool.tile([P, NJB * D], f32)
            gam_sb = cpool.tile([P, D], f32)
            abuf = cpool.tile([P, NJB * D], f32)
            tsb = cpool.tile([P, P], f32)

            nc.sync.dma_start(xi_rep[:], xi_rep_d[:])
            nc.sync.dma_start(sc2xj[:], sc2xj_d[:])
            nc.sync.dma_start(bias_sb[:], bias_d[:])
            nc.sync.dma_start(gam_sb[:], gam_d[:])

            gram = pspool.tile([P, P], f32)

            mm = 0
            for jb in range(NJB):
                # K stored i-major: column i*D + c, so gram operands are
                # contiguous [P, D*GJ] slices (matmul rhs needs 1 free dim).
                ktile = kpool.tile([P, D * NI], f32)
                k3 = ktile[:].rearrange("p (i c) -> p i c", c=D)
                for c in range(D):
                    e = epool.tile([P, NI], f32)
                    xi_c = xi_rep[:, c * NI : (c + 1) * NI]
                    col = jb * D + c
                    nc.vector.scalar_tensor_tensor(
                        out=e[:],
                        in0=xi_c,
                        scalar=sc2xj[:, col : col + 1],
                        in1=xi_c,
                        op0=mybir.AluOpType.subtract,
                        op1=mybir.AluOpType.mult,
                    )
                    nc.scalar.activation(
                        out=k3[:, :, c],
                        in_=e[:],
                        func=mybir.ActivationFunctionType.Exp,
                        bias=bias_sb[:, col : col + 1],
                        scale=gam_sb[:, c : c + 1],
                        accum_out=abuf[:, col : col + 1],
                    )
                for g in range(NI // GJ):
                    op = ktile[:, g * D * GJ : (g + 1) * D * GJ]
                    nc.tensor.matmul(
                        gram[:],
                        lhsT=op,
                        rhs=op,
                        start=(mm == 0),
                        stop=(mm == n_mm - 1),
                    )
                    mm += 1

            nc.vector.tensor_copy(tsb[:], gram[:])
            nc.sync.dma_start(apart_d[:], abuf[:])
            nc.sync.dma_start(tpart_d[:], tsb[:])
    return nc


def _get_nc():
    if "nc" not in _NC_CACHE:
        _NC_CACHE["nc"] = _build_nc()
    return _NC_CACHE["nc"]


def _make_in_maps(X):
    Xd = X.astype(np.float64)
    meanD = 2.0 * (np.mean(Xd * Xd, axis=0) - np.mean(Xd, axis=0) ** 2)  # [D]
    gamma = (1.0 / (2.0 * meanD)).astype(np.float32)  # 1/(2*sigma^2)

    xsq32 = X * X  # fl(x*x), matches device rounding
    bias_full = -(gamma[None, :] * xsq32)  # [N, D] f32
    sc2xj_full = 2.0 * X  # [N, D] f32

    def jblocked(a):  # [N, D] -> [P, NJB*D] with col jb*D+c = a[jb*128+p, c]
        return np.ascontiguousarray(
            a.reshape(NJB, P, D).transpose(1, 0, 2).reshape(P, NJB * D)
        )

    sc2xj = jblocked(sc2xj_full.astype(np.float32))
    bias_t = jblocked(bias_full.astype(np.float32))
    gam_t = np.ascontiguousarray(np.broadcast_to(-gamma[None, :], (P, D)))

    in_maps = []
    for r in range(NCORES):
        xi = X[r * NI : (r + 1) * NI, :]  # [NI, D]
        xi_rep = np.ascontiguousarray(
            np.broadcast_to(xi.T.reshape(1, D * NI), (P, D * NI))
        ).astype(np.float32)
        in_maps.append(
            {
                "xi_rep": xi_rep,
                "sc2xj": sc2xj,
                "bias_t": bias_t,
                "gam_t": gam_t.astype(np.float32),
            }
        )
    return in_maps


def _combine(results):
    A = np.zeros((D, N), dtype=np.float64)
    Tp = np.zeros((D, D), dtype=np.float64)
    for r in range(NCORES):
        ap = results[r]["apart"].astype(np.float64)  # [P, NJB*D]
        A += ap.reshape(P, NJB, D).transpose(2, 1, 0).reshape(D, N)
        tp = results[r]["tpart"].astype(np.float64).reshape(GJ, D, GJ, D)
        Tp += np.einsum("iaib->ab", tp)
    A -= 1.0  # remove diagonal K_ii = 1
    T = Tp - N  # remove sum_i K_ii^2
    S = A.sum(axis=1)
    Dm = A @ A.T
    c0 = 1.0 / (N * (N - 3))
    hsic = c0 * (
        T + np.outer(S, S) / ((N - 1.0) * (N - 2.0)) - (2.0 / (N - 2.0)) * Dm
    )
    iu = np.triu_indices(D, 1)
    return np.float32(np.sum(hsic[iu] ** 2))


def run_spmd(in_maps, **kwargs):
    from concourse import bass_utils

    nc = _get_nc()
    return bass_utils.run_bass_kernel_spmd(
        nc, in_maps, core_ids=list(range(NCORES)), **kwargs
    )


def kernel(X):
    X = np.ascontiguousarray(np.asarray(X, dtype=np.float32))
    in_maps = _make_in_maps(X)
    res = run_spmd(in_maps)
    return _combine(res.results)

